# revision 25
# baseline (speedup 1.0000x reference)
"""Associative-embedding loss kernel for 8 Trainium2 NeuronCores.

Math: per image b, with tl[n,c] = pred[b,c,ty,tx] and br[n,c] = target[b,c,by,bx]
gathered at the N=128 match points:
  pull_b = sum_{n,c} (tl-br)^2 / (2N)
  s'[n]  = sum_c (tl+br),  A'[i,j] = s'[i]-s'[j]   (A = A'/2)
  push_b = (0.5*(sum|A'+2| - sum|A'|) - N) / (N(N-1))
using sum_{ij} relu(1-|A|) = sum|A+1| - sum|A| for antisymmetric A.

Strategy: data-parallel over B (8 images per core). The host shards each
core's 128x2 match points into three small uploads (~320KB/core); HW
indirect DMA is limited to one index per partition per instruction
(~1.3us each, 16 per core), which made an on-device gather the dominant
cost, so the point extraction happens host-side and every loss FLOP runs
on device.

The corner/channel sums that produce s' are folded into the pairwise
matmul contraction (K=128, bf16):
  lhsT rows 8b+q       = raw values v[b, i, q]  (q = 8 corner x channel)
  lhsT rows 64+..      = -1
  rhs rows 8b+q        = 1 on column block b (constant indicator)
  rhs rows 64+8b+q     = v[b, j, q] on column block b, zeros elsewhere
  => out[i, 128b+j] = sum_q v[b,i,q] - sum_q v[b,j,q] = s'_b[i] - s'_b[j]
for all 8 images into one two-bank PSUM tile [128, 1024]. The Scalar
engine accumulates |A'+2| in one pass (Abs with bias via accum_out), the
Vector engine row-reduces |A'| in one pass, and pull comes from an fp32
subtract (GpSimd, 32 partitions) + square-accumulate (DVE) on a separate
[64, 128] upload. rh is split by column halves across two DMA queues so
the first matmul starts half a transfer earlier. bf16 rounding only
perturbs s' by ~0.4%, far inside the 2e-2 gate; pull stays fp32 exact.
Each core returns [128, 8] partial sums folded on the host in fp64.
"""

import numpy as np

B, C, H, W, N = 64, 4, 256, 256, 128
M = 8            # cores
BL = B // M      # images per core
Q = 2 * C        # corner x channel values per point

_GRAPH = None

# constant indicator rows: row 8b+q is 1 on column block b
_IND = np.repeat(np.kron(np.eye(8), np.ones((1, N))), Q, axis=0)


def _build_graph():
    import concourse.bacc as bacc
    import concourse.mybir as mybir
    from concourse.tile import TileContext

    f32 = mybir.dt.float32
    bf16 = mybir.dt.bfloat16
    Alu = mybir.AluOpType
    Act = mybir.ActivationFunctionType
    Axis = mybir.AxisListType

    nc = bacc.Bacc()
    lt_d = nc.declare_dram_parameter("lt", [128, 128], bf16, isOutput=False)
    rh_d = nc.declare_dram_parameter("rh", [128, 8 * N], bf16, isOutput=False)
    g_d = nc.declare_dram_parameter("g", [64, N], f32, isOutput=False)
    out_d = nc.declare_dram_parameter("out", [128, 8], f32, isOutput=True)

    with TileContext(nc) as tc:
        with (
            tc.tile_pool(name="sb", bufs=1) as pool,
            tc.tile_pool(name="ps", bufs=2, space="PSUM") as psum,
        ):
            # rh halves ride both HWDGE queues so each matmul's gate arrives
            # half a transfer earlier; g and lt share the queues' second slots
            rht = pool.tile([128, 8 * N], bf16)
            nc.sync.dma_start(out=rht[:, 0:512], in_=rh_d[:, 0:512])
            nc.scalar.dma_start(out=rht[:, 512:1024], in_=rh_d[:, 512:1024])
            gtl = pool.tile([32, N], f32)
            nc.sync.dma_start(out=gtl[:], in_=g_d[0:32, :])
            gbr = pool.tile([32, N], f32)
            nc.sync.dma_start(out=gbr[:], in_=g_d[32:64, :])
            ltt = pool.tile([128, 128], bf16)
            nc.scalar.dma_start(out=ltt[:], in_=lt_d[:])

            acc = pool.tile([128, 8], f32)
            nc.vector.memset(acc[:], 0.0)
            two = pool.tile([128, 1], f32)
            nc.vector.memset(two[:], 2.0)

            # pull: d = tl - br on GpSimd (rows 0-31 minus rows 32-63), then
            # square+accumulate on DVE -> acc[0:32, 0]
            dt_ = pool.tile([32, N], f32)
            nc.gpsimd.tensor_sub(dt_[:], gtl[:], gbr[:])
            d2 = pool.tile([32, N], f32)
            nc.vector.scalar_tensor_tensor(
                out=d2[:], in0=dt_[:], scalar=0.0, in1=dt_[:],
                op0=Alu.bypass, op1=Alu.mult, accum_out=acc[0:32, 0:1])

            # A'[i, 128b+j] = s'_b[i] - s'_b[j], all 8 images in one
            # two-bank PSUM tile
            bank = psum.tile([128, 8 * N], f32, name="bank", tag="a")
            nc.tensor.matmul(out=bank[:, 0:512], lhsT=ltt[:],
                             rhs=rht[:, 0:512], start=True, stop=True)
            nc.tensor.matmul(out=bank[:, 512:1024], lhsT=ltt[:],
                             rhs=rht[:, 512:1024], start=True, stop=True)

            # acc col1 = rowsum |A'+2|; col3 = rowsum |A'|
            scr = pool.tile([128, 8 * N], f32)
            nc.scalar.activation(
                out=scr[:], in_=bank[:], func=Act.Abs, bias=two[:, 0:1],
                scale=1.0, accum_out=acc[:, 1:2])
            nc.vector.tensor_reduce(
                out=acc[:, 3:4], in_=bank[:], axis=Axis.X, op=Alu.add,
                apply_absolute_value=True)

            nc.sync.dma_start(out=out_d[:], in_=acc[:])
    nc.finalize()
    return nc


def _get_graph():
    global _GRAPH
    if _GRAPH is None:
        _GRAPH = _build_graph()
    return _GRAPH


def _make_in_maps(pred, target, match):
    import ml_dtypes

    bf16 = ml_dtypes.bfloat16
    barr = np.arange(B)[:, None]
    tl = pred[barr, :, match[:, :, 0, 0], match[:, :, 0, 1]]    # [B, N, C]
    br = target[barr, :, match[:, :, 1, 0], match[:, :, 1, 1]]  # [B, N, C]
    raw = np.concatenate([tl, br], axis=-1)                     # [B, N, Q]
    raw16 = raw.astype(bf16)

    in_maps = []
    for i in range(M):
        sl = slice(i * BL, (i + 1) * BL)
        rc = raw16[sl]                                          # [BL, N, Q]
        lt = np.empty((128, 128), bf16)
        lt[0:64] = rc.transpose(0, 2, 1).reshape(64, N)         # rows 8b+q
        lt[64:128] = bf16(-1.0)
        rh = np.zeros((128, 8 * N), bf16)
        rh[0:64] = _IND
        for b in range(BL):
            rh[64 + Q * b:64 + Q * (b + 1), N * b:N * (b + 1)] = \
                rc[b].transpose(1, 0)
        # g rows 4b+c = tl[b, :, c]; rows 32+4b+c = br[b, :, c]
        g = np.empty((64, N), np.float32)
        g[0:32] = tl[sl].transpose(0, 2, 1).reshape(32, N)
        g[32:64] = br[sl].transpose(0, 2, 1).reshape(32, N)
        in_maps.append({"lt": lt, "rh": rh, "g": g})
    return in_maps


def _finish(core_outs):
    pull_total = 0.0
    m_total = 0.0
    for o in core_outs:
        o = np.asarray(o, dtype=np.float64)
        pull_total += o[:, 0].sum()
        m_total += (o[:, 1] + o[:, 2] - o[:, 3] - o[:, 4]).sum()
    # per image: 0.5*(sum|A'+2| - sum|A'|) = P_b + N
    pull_all = 0.25 * pull_total / (2 * N)
    push_all = 0.25 * (0.5 * m_total - B * N) / (N * (N - 1))
    return (np.float32(pull_all), np.float32(push_all))


def kernel(pred, target, match):
    from concourse.bass_utils import run_bass_kernel_spmd

    nc = _get_graph()
    in_maps = _make_in_maps(np.asarray(pred), np.asarray(target), np.asarray(match))
    res = run_bass_kernel_spmd(nc, in_maps, core_ids=list(range(M)))
    return _finish([r["out"] for r in res.results])


# revision 29
# speedup vs baseline: 1.1950x; 1.1950x over previous
"""Associative-embedding loss kernel for 8 Trainium2 NeuronCores.

Math: per image b, with tl[n,c] = pred[b,c,ty,tx] and br[n,c] = target[b,c,by,bx]
gathered at the N=128 match points:
  pull_b = sum_{n,c} (tl-br)^2 / (2N)
  s'[n]  = sum_c (tl+br),  A'[i,j] = s'[i]-s'[j]   (A = A'/2)
  push_b = (0.5*(sum|A'+2| - sum|A'|) - N) / (N(N-1))
using sum_{ij} relu(1-|A|) = sum|A+1| - sum|A| for antisymmetric A.

Strategy: data-parallel over B (8 images per core). The host shards each
core's 128x2 match points into three small uploads (~320KB/core); HW
indirect DMA is limited to one index per partition per instruction
(~1.3us each, 16 per core), which made an on-device gather the dominant
cost, so the point extraction happens host-side and every loss FLOP runs
on device.

The corner/channel sums that produce s' are folded into the pairwise
matmul contraction (K=128, bf16):
  lhsT rows 8b+q       = raw values v[b, i, q]  (q = 8 corner x channel)
  lhsT rows 64+..      = -1
  rhs rows 8b+q        = 1 on column block b (constant indicator)
  rhs rows 64+8b+q     = v[b, j, q] on column block b, zeros elsewhere
  => out[i, 128b+j] = sum_q v[b,i,q] - sum_q v[b,j,q] = s'_b[i] - s'_b[j]
for all 8 images into one two-bank PSUM tile [128, 1024]. The Scalar
engine accumulates |A'+2| in one pass (Abs with bias via accum_out), the
Vector engine row-reduces |A'| in one pass, and pull comes from an fp32
subtract (GpSimd, 32 partitions) + square-accumulate (DVE) on a separate
[64, 128] upload. rh is split by column halves across two DMA queues so
the first matmul starts half a transfer earlier. bf16 rounding only
perturbs s' by ~0.4%, far inside the 2e-2 gate; pull stays fp32 exact.
Each core returns [128, 8] partial sums folded on the host in fp64.
"""

import numpy as np

B, C, H, W, N = 64, 4, 256, 256, 128
M = 8            # cores
BL = B // M      # images per core
Q = 2 * C        # corner x channel values per point

_GRAPH = None

# constant indicator rows: row 8b+q is 1 on column block b
_IND = np.repeat(np.kron(np.eye(8), np.ones((1, N))), Q, axis=0)


def _build_graph():
    import concourse.bacc as bacc
    import concourse.mybir as mybir
    from concourse.tile import TileContext

    f32 = mybir.dt.float32
    bf16 = mybir.dt.bfloat16
    Alu = mybir.AluOpType
    Act = mybir.ActivationFunctionType
    Axis = mybir.AxisListType

    nc = bacc.Bacc()
    lt_d = nc.declare_dram_parameter("lt", [128, 128], bf16, isOutput=False)
    rh_d = nc.declare_dram_parameter("rh", [128, 8 * N], bf16, isOutput=False)
    g_d = nc.declare_dram_parameter("g", [32, 2 * N], f32, isOutput=False)
    out_d = nc.declare_dram_parameter("out", [128, 8], f32, isOutput=True)

    with TileContext(nc) as tc:
        with (
            tc.tile_pool(name="sb", bufs=1) as pool,
            tc.tile_pool(name="ps", bufs=2, space="PSUM") as psum,
        ):
            # rh (the matmul gate, largest) first on the sync queue; lt first
            # on the scalar queue so LDWEIGHTS is never the straggler
            rht = pool.tile([128, 8 * N], bf16)
            nc.sync.dma_start(out=rht[:], in_=rh_d[:])
            ltt = pool.tile([128, 128], bf16)
            nc.scalar.dma_start(out=ltt[:], in_=lt_d[:])
            g = pool.tile([32, 2 * N], f32)
            nc.scalar.dma_start(out=g[:], in_=g_d[:])

            acc = pool.tile([128, 8], f32)
            nc.vector.memset(acc[:], 0.0)
            two = pool.tile([128, 1], f32)
            nc.vector.memset(two[:], 2.0)

            # pull: d = tl - br on GpSimd (rows 0-31 minus rows 32-63), then
            # square+accumulate on DVE -> acc[0:32, 0]
            dt_ = pool.tile([32, N], f32)
            nc.gpsimd.tensor_sub(dt_[:], g[:, 0:N], g[:, N:2 * N])
            d2 = pool.tile([32, N], f32)
            nc.vector.scalar_tensor_tensor(
                out=d2[:], in0=dt_[:], scalar=0.0, in1=dt_[:],
                op0=Alu.bypass, op1=Alu.mult, accum_out=acc[0:32, 0:1])

            # A'[i, 128b+j] = s'_b[i] - s'_b[j], all 8 images in one
            # two-bank PSUM tile
            bank = psum.tile([128, 8 * N], f32, name="bank", tag="a")
            nc.tensor.matmul(out=bank[:, 0:512], lhsT=ltt[:],
                             rhs=rht[:, 0:512], start=True, stop=True)
            nc.tensor.matmul(out=bank[:, 512:1024], lhsT=ltt[:],
                             rhs=rht[:, 512:1024], start=True, stop=True)

            # acc col1 = rowsum |A'+2|; col3 = rowsum |A'|
            scr = pool.tile([128, 8 * N], f32)
            nc.scalar.activation(
                out=scr[:], in_=bank[:], func=Act.Abs, bias=two[:, 0:1],
                scale=1.0, accum_out=acc[:, 1:2])
            nc.vector.tensor_reduce(
                out=acc[:, 3:4], in_=bank[:], axis=Axis.X, op=Alu.add,
                apply_absolute_value=True)

            nc.sync.dma_start(out=out_d[:], in_=acc[:])
    nc.finalize()
    return nc


def _get_graph():
    global _GRAPH
    if _GRAPH is None:
        _GRAPH = _build_graph()
    return _GRAPH


def _make_in_maps(pred, target, match):
    import ml_dtypes

    bf16 = ml_dtypes.bfloat16
    barr = np.arange(B)[:, None]
    tl = pred[barr, :, match[:, :, 0, 0], match[:, :, 0, 1]]    # [B, N, C]
    br = target[barr, :, match[:, :, 1, 0], match[:, :, 1, 1]]  # [B, N, C]
    raw = np.concatenate([tl, br], axis=-1)                     # [B, N, Q]
    raw16 = raw.astype(bf16)

    in_maps = []
    for i in range(M):
        sl = slice(i * BL, (i + 1) * BL)
        rc = raw16[sl]                                          # [BL, N, Q]
        lt = np.empty((128, 128), bf16)
        lt[0:64] = rc.transpose(0, 2, 1).reshape(64, N)         # rows 8b+q
        lt[64:128] = bf16(-1.0)
        rh = np.zeros((128, 8 * N), bf16)
        rh[0:64] = _IND
        for b in range(BL):
            rh[64 + Q * b:64 + Q * (b + 1), N * b:N * (b + 1)] = \
                rc[b].transpose(1, 0)
        # g row 4b+c = [tl[b, :, c] | br[b, :, c]]
        g = np.empty((32, 2 * N), np.float32)
        g[:, 0:N] = tl[sl].transpose(0, 2, 1).reshape(32, N)
        g[:, N:2 * N] = br[sl].transpose(0, 2, 1).reshape(32, N)
        in_maps.append({"lt": lt, "rh": rh, "g": g})
    return in_maps


def _finish(core_outs):
    pull_total = 0.0
    m_total = 0.0
    for o in core_outs:
        o = np.asarray(o, dtype=np.float64)
        pull_total += o[:, 0].sum()
        m_total += (o[:, 1] + o[:, 2] - o[:, 3] - o[:, 4]).sum()
    # per image: 0.5*(sum|A'+2| - sum|A'|) = P_b + N
    pull_all = 0.25 * pull_total / (2 * N)
    push_all = 0.25 * (0.5 * m_total - B * N) / (N * (N - 1))
    return (np.float32(pull_all), np.float32(push_all))


def kernel(pred, target, match):
    from concourse.bass_utils import run_bass_kernel_spmd

    nc = _get_graph()
    in_maps = _make_in_maps(np.asarray(pred), np.asarray(target), np.asarray(match))
    res = run_bass_kernel_spmd(nc, in_maps, core_ids=list(range(M)))
    return _finish([r["out"] for r in res.results])
